# revision 1
# baseline (speedup 1.0000x reference)
"""MoE routing kernel (nn_Bool_40793599377512) for 8 trn2 NeuronCores.

out[n] = tanh(x[n] @ W[g(n)] + b[g(n)]),  g(n) = (mean(x[n]) > 0)

Strategy (expert-parallel): route rows on the host (cheap: one mean per
row), give each core a slice of rows that all use ONE expert, and run a
dense  y.T = W_e.T @ x_slice.T  matmul per core with fp32r (FP22) PE
passes at full bf16-rate. Bias+tanh are fused into one ScalarE
activation per output tile. Rows are padded per-core to a fixed capacity
C so the compiled program is input-independent.
"""

import functools
import os
import sys
from contextlib import ExitStack

import numpy as np

for _p in ("/opt/trn_rl_repo", "/root/.axon_site/_ro/trn_rl_repo"):
    if os.path.isdir(_p) and _p not in sys.path:
        sys.path.append(_p)

import concourse.bacc as bacc
import concourse.tile as tile
from concourse import mybir
from concourse.bass_utils import run_bass_kernel_spmd


def _ensure_axon_ntff_hook():
    """Register the NTFF-profile hook that bass_utils expects under axon.

    This image's ``antenv`` package lacks ``axon_hooks``; without it,
    ``run_bass_kernel_spmd(trace=True)`` (e.g. via BASS_TRACE=1) crashes
    on import instead of profiling. Provide the module and wire in the
    ctypes hook from the axon boot shim when available.
    """
    try:
        import antenv.axon_hooks  # noqa: F401

        return
    except ImportError:
        pass
    try:
        import types

        import antenv

        mod = types.ModuleType("antenv.axon_hooks")
        state = {"hook": None}
        mod.set_axon_ntff_profile_hook = lambda h: state.__setitem__("hook", h)
        mod.get_axon_ntff_profile_hook = lambda: state["hook"]
        sys.modules["antenv.axon_hooks"] = mod
        antenv.axon_hooks = mod
        if "/root/.axon_site" not in sys.path:
            sys.path.append("/root/.axon_site")
        from trn_agent_boot.trn_boot import _ntff_profile_via_ctypes

        hook = _ntff_profile_via_ctypes("/opt/axon/libaxon_pjrt.so")
        if hook is not None:
            mod.set_axon_ntff_profile_hook(hook)
    except Exception:
        pass


_ensure_axon_ntff_hook()

N_TOK, D_IN, D_OUT, N_EXPERTS, NCORES = 8192, 4096, 4096, 2, 8
P = 128
F32 = mybir.dt.float32
F32R = mybir.dt.float32r

LAST_RUN = None  # BassKernelResults of the most recent hardware run


def _chunks(c):
    """Split token count c into balanced matmul N-chunks.

    fp32r runs at full PE rate only for N >= 256 (and N <= 512 is the
    fp32 moving-operand max). Equal-sized chunks measured lower
    LDWEIGHTS exposure than a maximal-512 greedy split.
    """
    n = -(-c // 512)
    q, tail = divmod(c, 8)
    units = [q // n + (1 if j < q % n else 0) for j in range(n)]
    out = [8 * u for u in units]
    out[-1] += tail  # c is snapped to 16 in practice, so tail == 0
    return out


@functools.lru_cache(maxsize=4)
def _build(c_cap, d_in=D_IN, d_out=D_OUT):
    """Build + compile the per-core Bass program (same for all 8 cores).

    Inputs per core: xT [d_in, c_cap] f32 (tokens pre-transposed on
    host), W [d_in, d_out] f32 (this core's expert), bT [128, d_out/128]
    f32 (bias regrouped per m-chunk). Output: yT [d_out, c_cap].
    """
    kt = d_in // P   # K tiles (contraction)
    mt = d_out // P  # output-row tiles
    chunks = _chunks(c_cap)

    nc = bacc.Bacc(
        "TRN2", target_bir_lowering=False, debug=False, num_devices=NCORES
    )
    xT = nc.dram_tensor("xT", [d_in, c_cap], F32R, kind="ExternalInput").ap()
    Wd = nc.dram_tensor("W", [d_in, d_out], F32R, kind="ExternalInput").ap()
    bd = nc.dram_tensor("bT", [P, mt], F32, kind="ExternalInput").ap()
    yT = nc.dram_tensor("yT", [d_out, c_cap], F32, kind="ExternalOutput").ap()

    # SBUF (KB/partition): x.T resident + W columns + output staging.
    # Fit 3-deep W prefetch when possible (overlaps the initial x.T
    # sweep with up to 3 matmul columns), else fall back to 2.
    xt_kb = kt * c_cap * 4 / 1024
    w_col_kb = kt * P * 4 / 1024
    out_kb = 2 * c_cap * 4 / 1024
    w_bufs = 3 if xt_kb + 3 * w_col_kb + out_kb + 1 <= 189 else 2

    # PSUM: one bank per chunk-psum; spread the 8 banks over the
    # chunk tags so 2-3 output columns can accumulate concurrently.
    n_ch = len(chunks)
    ps_bufs = [8 // n_ch + (1 if j < 8 % n_ch else 0) for j in range(n_ch)]
    ps_bufs = [min(b, 4) for b in ps_bufs]

    with tile.TileContext(nc) as tc:
        with ExitStack() as ctx:
            n_seg = 4 if kt % 4 == 0 else 1
            seg_k = kt // n_seg  # k-tiles per W segment

            xt_pool = ctx.enter_context(tc.tile_pool(name="xt", bufs=1))
            w_pool = ctx.enter_context(
                tc.tile_pool(name="w", bufs=w_bufs * n_seg)
            )
            ps_pool = ctx.enter_context(
                tc.tile_pool(name="ps", bufs=1, space="PSUM")
            )
            out_pool = ctx.enter_context(tc.tile_pool(name="out", bufs=2))
            b_pool = ctx.enter_context(tc.tile_pool(name="b", bufs=1))


            # W viewed as [p, k, m]; each m-column is fetched as n_seg
            # k-segments so (a) the very first matmul only waits for a
            # ~512KB segment, and (b) segment slots recycle mid-column,
            # keeping the W prefetch pipeline full with no post-sweep
            # bubble. The first w_bufs columns interleave with the EARLY
            # x.T k-tiles so the PE runs 2-3 output columns while x.T
            # streams in.
            W_r = Wd.rearrange("(k p) m -> p k m", p=P)
            xt_all = xt_pool.tile([P, kt * c_cap], F32R)

            def load_w_seg(m, s):
                wt = w_pool.tile(
                    [P, seg_k * P], F32R, name=f"wt{m}_{s}", tag="wt"
                )
                nc.sync.dma_start(
                    wt[:].rearrange("p (k c) -> p k c", k=seg_k),
                    W_r[:, s * seg_k : (s + 1) * seg_k, m * P : (m + 1) * P],
                )
                return wt

            def load_w(m):
                return [load_w_seg(m, s) for s in range(n_seg)]

            def load_xt(k):
                nc.sync.dma_start(
                    xt_all[:, k * c_cap : (k + 1) * c_cap],
                    xT[k * P : (k + 1) * P, :],
                )

            # NOTE: deliberately let the x.T stream run AHEAD of the
            # PE (W column 0 first, then x.T k-tiles): starting the PE
            # earlier just fragments its work while DMA-bound and
            # bounces the HAM clock gate (measured +6us).
            bias_t = b_pool.tile([P, mt], F32)
            nc.sync.dma_start(bias_t[:], bd)
            w_head = min(w_bufs, mt)
            wts = {}
            for i in range(w_head):
                wts[i] = load_w(i)
                load_xt(i)
            for k in range(w_head, kt):
                load_xt(k)

            for m in range(mt):
                wsegs = wts.pop(m) if m in wts else load_w(m)
                psums = [
                    ps_pool.tile(
                        [P, ch],
                        F32,
                        tag=f"ps{j}",
                        name=f"ps{j}_{m}",
                        bufs=ps_bufs[j],
                    )
                    for j, ch in enumerate(chunks)
                ]
                for k in range(kt):
                    wt = wsegs[k // seg_k]
                    kc = k % seg_k
                    off = 0
                    for j, ch in enumerate(chunks):
                        nc.tensor.matmul(
                            psums[j][:],
                            wt[:, kc * P : (kc + 1) * P],
                            xt_all[
                                :, k * c_cap + off : k * c_cap + off + ch
                            ],
                            start=(k == 0),
                            stop=(k == kt - 1),
                        )
                        off += ch
                out_t = out_pool.tile([P, c_cap], F32)
                off = 0
                for j, ch in enumerate(chunks):
                    nc.scalar.activation(
                        out_t[:, off : off + ch],
                        psums[j][:],
                        mybir.ActivationFunctionType.Tanh,
                        bias=bias_t[:, m : m + 1],
                    )
                    off += ch
                nc.sync.dma_start(yT[m * P : (m + 1) * P, :], out_t[:])
    nc.compile()
    return nc


def _route(x):
    """Expert id per row, matching the reference's (mean(x,-1) > 0)."""
    # float64 accumulation: any fp32 summation order agrees with this
    # sign unless |mean| is within ~1e-9 of zero (never for randn data).
    return (x.astype(np.float64).mean(axis=1) > 0.0).astype(np.int32)


def _core_assignment(counts):
    """Number of cores per expert minimizing the max per-core row load."""
    best = None
    for c0 in range(NCORES + 1):
        c1 = NCORES - c0
        if (counts[0] > 0 and c0 == 0) or (counts[1] > 0 and c1 == 0):
            continue
        load = 0
        if c0:
            load = max(load, -(-counts[0] // c0))
        if c1:
            load = max(load, -(-counts[1] // c1))
        if best is None or load < best[0]:
            best = (load, c0, c1)
    return best


def kernel(x, W, b):
    global LAST_RUN
    x = np.ascontiguousarray(x, dtype=np.float32)
    W = np.ascontiguousarray(W, dtype=np.float32)
    b = np.ascontiguousarray(b, dtype=np.float32)
    n_tok, d_in = x.shape
    d_out = W.shape[2]
    mt = d_out // P

    g = _route(x)
    idx = [np.nonzero(g == e)[0] for e in range(N_EXPERTS)]
    load, c0, c1 = _core_assignment([len(idx[0]), len(idx[1])])
    c_cap = max(256, -(-load // 16) * 16)

    nc = _build(c_cap, d_in, d_out)

    # Pre-transpose x once; per-core slices are column gathers.
    xT_full = np.ascontiguousarray(x.T)

    groups = []  # per core: (expert, row-index array)
    for e, ncr in ((0, c0), (1, c1)):
        if ncr:
            groups.extend((e, part) for part in np.array_split(idx[e], ncr))
    assert len(groups) == NCORES

    bT = [np.ascontiguousarray(b[e].reshape(mt, P).T) for e in range(N_EXPERTS)]
    in_maps = []
    for e, rows in groups:
        xTc = np.zeros((d_in, c_cap), dtype=np.float32)
        if len(rows):
            np.take(xT_full, rows, axis=1, out=xTc[:, : len(rows)])
        in_maps.append({"xT": xTc, "W": W[e], "bT": bT[e]})

    res = run_bass_kernel_spmd(nc, in_maps, core_ids=list(range(NCORES)))
    LAST_RUN = res

    y = np.empty((n_tok, d_out), dtype=np.float32)
    for (e, rows), core_out in zip(groups, res.results):
        if len(rows):
            y[rows] = core_out["yT"][:, : len(rows)].T
    return y



# revision 2
# speedup vs baseline: 1.1287x; 1.1287x over previous
"""MoE routing kernel (nn_Bool_40793599377512) for 8 trn2 NeuronCores.

out[n] = tanh(x[n] @ W[g(n)] + b[g(n)]),  g(n) = (mean(x[n]) > 0)

Strategy (expert-parallel): route rows on the host (cheap: one mean per
row), give each core a slice of rows that all use ONE expert, and run a
dense  y.T = W_e.T @ x_slice.T  matmul per core. The PE runs at 1
col/cycle for any 16-bit or fp8 MOVING operand, so precision is spent
where it's free: W stationary in bf16 (halves W DMA vs fp32), x moving
in fp8-e3m4 (quarter DMA -> the first output column never waits on the
x stream), y written back as bf16. Measured end-to-end rel err 1.5e-2
(gate 2e-2); with XDT=bf16 it is 2.9e-3.

W is pre-blocked on the host into [m-col, k-seg, partition, bytes] so
every W DMA is a fully contiguous 2KB-per-partition transfer. Rows are
padded per-core to a fixed capacity so the compiled program is
input-independent.
"""

import functools
import os
import sys
from contextlib import ExitStack

import ml_dtypes
import numpy as np

for _p in ("/opt/trn_rl_repo", "/root/.axon_site/_ro/trn_rl_repo"):
    if os.path.isdir(_p) and _p not in sys.path:
        sys.path.append(_p)

import concourse.bacc as bacc
import concourse.tile as tile
from concourse import mybir
from concourse.bass_utils import run_bass_kernel_spmd


def _ensure_axon_ntff_hook():
    """Register the NTFF-profile hook that bass_utils expects under axon.

    This image's ``antenv`` package lacks ``axon_hooks``; without it,
    ``run_bass_kernel_spmd(trace=True)`` (e.g. via BASS_TRACE=1) crashes
    on import instead of profiling. Provide the module and wire in the
    ctypes hook from the axon boot shim when available.
    """
    try:
        import antenv.axon_hooks  # noqa: F401

        return
    except ImportError:
        pass
    try:
        import types

        import antenv

        mod = types.ModuleType("antenv.axon_hooks")
        state = {"hook": None}
        mod.set_axon_ntff_profile_hook = lambda h: state.__setitem__("hook", h)
        mod.get_axon_ntff_profile_hook = lambda: state["hook"]
        sys.modules["antenv.axon_hooks"] = mod
        antenv.axon_hooks = mod
        if "/root/.axon_site" not in sys.path:
            sys.path.append("/root/.axon_site")
        from trn_agent_boot.trn_boot import _ntff_profile_via_ctypes

        hook = _ntff_profile_via_ctypes("/opt/axon/libaxon_pjrt.so")
        if hook is not None:
            mod.set_axon_ntff_profile_hook(hook)
    except Exception:
        pass


_ensure_axon_ntff_hook()

N_TOK, D_IN, D_OUT, N_EXPERTS, NCORES = 8192, 4096, 4096, 2, 8
P = 128
F32 = mybir.dt.float32
BF16 = mybir.dt.bfloat16

# Moving-operand dtype for x: fp8-e3m4 (4-bit mantissa) streams 4x less
# than fp32 at the same PE rate; X_SCALE uses the e3m4 exponent range
# ([-16, 16)) for |x| up to ~5.4. Set XDT = BF16 / X_SCALE = 1.0 for the
# higher-accuracy variant.
XDT = mybir.dt.float8e3
XDT_NP = ml_dtypes.float8_e3m4
X_SCALE = 2.0

N_SEG = 4          # W column is fetched as 4 K-segments
SEG_K = D_IN // P // N_SEG

LAST_RUN = None  # BassKernelResults of the most recent hardware run


def _chunks(c):
    """Split token count c into balanced matmul N-chunks (<=512 each)."""
    n = -(-c // 512)
    q, tail = divmod(c, 8)
    units = [q // n + (1 if j < q % n else 0) for j in range(n)]
    out = [8 * u for u in units]
    out[-1] += tail  # c is snapped to 16 in practice, so tail == 0
    return out


@functools.lru_cache(maxsize=4)
def _build(c_cap, d_in=D_IN, d_out=D_OUT):
    """Build + compile the per-core Bass program (same for all 8 cores).

    Inputs per core: xT [d_in, c_cap] fp8/bf16 (tokens pre-transposed +
    quantized on host), W [mt, N_SEG, P, SEG_K*P] bf16 (this core's
    expert, pre-blocked per m-column/K-segment), bT [128, d_out/128] f32
    (bias regrouped per m-chunk). Output: yT [d_out, c_cap] bf16.
    """
    kt = d_in // P   # K tiles (contraction)
    mt = d_out // P  # output-row tiles
    chunks = _chunks(c_cap)

    nc = bacc.Bacc(
        "TRN2", target_bir_lowering=False, debug=False, num_devices=NCORES
    )
    xT = nc.dram_tensor("xT", [d_in, c_cap], XDT, kind="ExternalInput").ap()
    Wd = nc.dram_tensor(
        "W", [mt, N_SEG, P, SEG_K * P], BF16, kind="ExternalInput"
    ).ap()
    bd = nc.dram_tensor("bT", [P, mt], F32, kind="ExternalInput").ap()
    yT = nc.dram_tensor("yT", [d_out, c_cap], BF16, kind="ExternalOutput").ap()

    w_bufs = 6

    # PSUM: one bank per chunk-psum; spread the 8 banks over the
    # chunk tags so 2-3 output columns can accumulate concurrently.
    n_ch = len(chunks)
    ps_bufs = [8 // n_ch + (1 if j < 8 % n_ch else 0) for j in range(n_ch)]
    ps_bufs = [min(b, 4) for b in ps_bufs]

    with tile.TileContext(nc) as tc:
        with ExitStack() as ctx:
            xt_pool = ctx.enter_context(tc.tile_pool(name="xt", bufs=1))
            w_pool = ctx.enter_context(
                tc.tile_pool(name="w", bufs=w_bufs * N_SEG)
            )
            ps_pool = ctx.enter_context(
                tc.tile_pool(name="ps", bufs=1, space="PSUM")
            )
            out_pool = ctx.enter_context(tc.tile_pool(name="out", bufs=2))
            b_pool = ctx.enter_context(tc.tile_pool(name="b", bufs=1))

            xt_all = xt_pool.tile([P, kt * c_cap], XDT)

            def load_w_seg(m, s):
                wt = w_pool.tile(
                    [P, SEG_K * P], BF16, name=f"wt{m}_{s}", tag="wt"
                )
                nc.sync.dma_start(wt[:], Wd[m, s])
                return wt

            def load_w(m):
                return [load_w_seg(m, s) for s in range(N_SEG)]

            def load_xt(k):
                nc.sync.dma_start(
                    xt_all[:, k * c_cap : (k + 1) * c_cap],
                    xT[k * P : (k + 1) * P, :],
                )

            # Startup: W col0 first (PE can begin at ~1us), then stream
            # x k-tiles interleaved with the next W columns so the PE
            # chases the x stream through col0 with minimal stalls.
            bias_t = b_pool.tile([P, mt], F32)
            nc.sync.dma_start(bias_t[:], bd)
            w_head = min(w_bufs, mt)
            wts = {0: load_w(0)}
            xk = 0
            for i, xk_target in zip(range(1, w_head), (4, 12, 20)):
                while xk < xk_target:
                    load_xt(xk)
                    xk += 1
                wts[i] = load_w(i)
            while xk < kt:
                load_xt(xk)
                xk += 1
            for i in range(len(wts), w_head):
                wts[i] = load_w(i)

            for m in range(mt):
                wsegs = wts.pop(m) if m in wts else load_w(m)
                psums = [
                    ps_pool.tile(
                        [P, ch],
                        F32,
                        tag=f"ps{j}",
                        name=f"ps{j}_{m}",
                        bufs=ps_bufs[j],
                    )
                    for j, ch in enumerate(chunks)
                ]
                for k in range(kt):
                    wt = wsegs[k // SEG_K]
                    kc = k % SEG_K
                    off = 0
                    for j, ch in enumerate(chunks):
                        nc.tensor.matmul(
                            psums[j][:],
                            wt[:, kc * P : (kc + 1) * P],
                            xt_all[
                                :, k * c_cap + off : k * c_cap + off + ch
                            ],
                            start=(k == 0),
                            stop=(k == kt - 1),
                        )
                        off += ch
                out_t = out_pool.tile([P, c_cap], BF16)
                off = 0
                for j, ch in enumerate(chunks):
                    nc.scalar.activation(
                        out_t[:, off : off + ch],
                        psums[j][:],
                        mybir.ActivationFunctionType.Tanh,
                        bias=bias_t[:, m : m + 1],
                        scale=float(1.0 / X_SCALE),
                    )
                    off += ch
                nc.sync.dma_start(yT[m * P : (m + 1) * P, :], out_t[:])
    nc.compile()
    return nc


def _route(x):
    """Expert id per row, matching the reference's (mean(x,-1) > 0)."""
    # float64 accumulation: any fp32 summation order agrees with this
    # sign unless |mean| is within ~1e-9 of zero (never for randn data).
    return (x.astype(np.float64).mean(axis=1) > 0.0).astype(np.int32)


def _core_assignment(counts):
    """Number of cores per expert minimizing the max per-core row load."""
    best = None
    for c0 in range(NCORES + 1):
        c1 = NCORES - c0
        if (counts[0] > 0 and c0 == 0) or (counts[1] > 0 and c1 == 0):
            continue
        load = 0
        if c0:
            load = max(load, -(-counts[0] // c0))
        if c1:
            load = max(load, -(-counts[1] // c1))
        if best is None or load < best[0]:
            best = (load, c0, c1)
    return best


def _preblock_w(We):
    """[d_in, d_out] f32 -> [mt, N_SEG, P, SEG_K*P] bf16 (contiguous
    2KB-per-partition DMA blocks: element [m, s, p, k*P + c] =
    W[(s*SEG_K + k)*P + p, m*P + c])."""
    d_in, d_out = We.shape
    mt = d_out // P
    Wb = We.astype(ml_dtypes.bfloat16)
    Wb = Wb.reshape(N_SEG, SEG_K, P, mt, P)
    return np.ascontiguousarray(Wb.transpose(3, 0, 2, 1, 4)).reshape(
        mt, N_SEG, P, SEG_K * P
    )


def kernel(x, W, b):
    global LAST_RUN
    x = np.ascontiguousarray(x, dtype=np.float32)
    W = np.ascontiguousarray(W, dtype=np.float32)
    b = np.ascontiguousarray(b, dtype=np.float32)
    n_tok, d_in = x.shape
    d_out = W.shape[2]
    mt = d_out // P

    g = _route(x)
    idx = [np.nonzero(g == e)[0] for e in range(N_EXPERTS)]
    load, c0, c1 = _core_assignment([len(idx[0]), len(idx[1])])
    c_cap = max(256, -(-load // 16) * 16)

    nc = _build(c_cap, d_in, d_out)

    # Quantize x once (scaled into the e3m4 sweet range), then gather
    # per-core column slices from the transposed copy.
    xq = (x * X_SCALE).astype(XDT_NP)
    xqT = np.ascontiguousarray(xq.T)

    groups = []  # per core: (expert, row-index array)
    for e, ncr in ((0, c0), (1, c1)):
        if ncr:
            groups.extend((e, part) for part in np.array_split(idx[e], ncr))
    assert len(groups) == NCORES

    Wblk = [_preblock_w(W[e]) for e in range(N_EXPERTS)]
    bT = [np.ascontiguousarray(b[e].reshape(mt, P).T) for e in range(N_EXPERTS)]
    in_maps = []
    for e, rows in groups:
        xTc = np.zeros((d_in, c_cap), dtype=XDT_NP)
        if len(rows):
            np.take(xqT, rows, axis=1, out=xTc[:, : len(rows)])
        in_maps.append({"xT": xTc, "W": Wblk[e], "bT": bT[e]})

    res = run_bass_kernel_spmd(nc, in_maps, core_ids=list(range(NCORES)))
    LAST_RUN = res

    y = np.empty((n_tok, d_out), dtype=np.float32)
    for (e, rows), core_out in zip(groups, res.results):
        if len(rows):
            y[rows] = core_out["yT"][:, : len(rows)].T.astype(np.float32)
    return y


# revision 9
# speedup vs baseline: 1.1301x; 1.0012x over previous
"""MoE routing kernel (nn_Bool_40793599377512) for 8 trn2 NeuronCores.

out[n] = tanh(x[n] @ W[g(n)] + b[g(n)]),  g(n) = (mean(x[n]) > 0)

Strategy (expert-parallel): route rows on the host (cheap: one mean per
row), give each core a slice of rows that all use ONE expert, and run a
dense  y.T = W_e.T @ x_slice.T  matmul per core. The PE runs at 1
col/cycle for any 16-bit or fp8 MOVING operand, so precision is spent
where it's free: W stationary in bf16 (halves W DMA vs fp32), x moving
in fp8-e3m4 (quarter DMA -> the first output column never waits on the
x stream), y written back as bf16. Measured end-to-end rel err 1.5e-2
(gate 2e-2); with XDT=bf16 it is 2.9e-3.

W is pre-blocked on the host into [m-col, k-seg, partition, bytes] so
every W DMA is a fully contiguous 2KB-per-partition transfer. Rows are
padded per-core to a fixed capacity so the compiled program is
input-independent.
"""

import functools
import os
import sys
from contextlib import ExitStack

import ml_dtypes
import numpy as np

for _p in ("/opt/trn_rl_repo", "/root/.axon_site/_ro/trn_rl_repo"):
    if os.path.isdir(_p) and _p not in sys.path:
        sys.path.append(_p)

import concourse.bacc as bacc
import concourse.tile as tile
from concourse import mybir
from concourse.bass_utils import run_bass_kernel_spmd


def _ensure_axon_ntff_hook():
    """Register the NTFF-profile hook that bass_utils expects under axon.

    This image's ``antenv`` package lacks ``axon_hooks``; without it,
    ``run_bass_kernel_spmd(trace=True)`` (e.g. via BASS_TRACE=1) crashes
    on import instead of profiling. Provide the module and wire in the
    ctypes hook from the axon boot shim when available.
    """
    try:
        import antenv.axon_hooks  # noqa: F401

        return
    except ImportError:
        pass
    try:
        import types

        import antenv

        mod = types.ModuleType("antenv.axon_hooks")
        state = {"hook": None}
        mod.set_axon_ntff_profile_hook = lambda h: state.__setitem__("hook", h)
        mod.get_axon_ntff_profile_hook = lambda: state["hook"]
        sys.modules["antenv.axon_hooks"] = mod
        antenv.axon_hooks = mod
        if "/root/.axon_site" not in sys.path:
            sys.path.append("/root/.axon_site")
        from trn_agent_boot.trn_boot import _ntff_profile_via_ctypes

        hook = _ntff_profile_via_ctypes("/opt/axon/libaxon_pjrt.so")
        if hook is not None:
            mod.set_axon_ntff_profile_hook(hook)
    except Exception:
        pass


_ensure_axon_ntff_hook()

N_TOK, D_IN, D_OUT, N_EXPERTS, NCORES = 8192, 4096, 4096, 2, 8
P = 128
F32 = mybir.dt.float32
BF16 = mybir.dt.bfloat16

# Moving-operand dtype for x: fp8-e3m4 (4-bit mantissa) streams 4x less
# than fp32 at the same PE rate; X_SCALE uses the e3m4 exponent range
# ([-16, 16)) for |x| up to ~5.4. Set XDT = BF16 / X_SCALE = 1.0 for the
# higher-accuracy variant.
XDT = mybir.dt.float8e3
XDT_NP = ml_dtypes.float8_e3m4
X_SCALE = 2.0

N_SEG = 4          # W column is fetched as 4 K-segments
SEG_K = D_IN // P // N_SEG

LAST_RUN = None  # BassKernelResults of the most recent hardware run


def _chunks(c):
    """Split token count c into balanced matmul N-chunks (<=512 each)."""
    n = -(-c // 512)
    q, tail = divmod(c, 8)
    units = [q // n + (1 if j < q % n else 0) for j in range(n)]
    out = [8 * u for u in units]
    out[-1] += tail  # c is snapped to 16 in practice, so tail == 0
    return out


@functools.lru_cache(maxsize=4)
def _build(c_cap, d_in=D_IN, d_out=D_OUT):
    """Build + compile the per-core Bass program (same for all 8 cores).

    Inputs per core: xT [d_in, c_cap] fp8/bf16 (tokens pre-transposed +
    quantized on host), W [mt, N_SEG, P, SEG_K*P] bf16 (this core's
    expert, pre-blocked per m-column/K-segment), bT [128, d_out/128] f32
    (bias regrouped per m-chunk). Output: yT [d_out, c_cap] bf16.
    """
    kt = d_in // P   # K tiles (contraction)
    mt = d_out // P  # output-row tiles
    chunks = _chunks(c_cap)

    nc = bacc.Bacc(
        "TRN2", target_bir_lowering=False, debug=False, num_devices=NCORES
    )
    xT = nc.dram_tensor("xT", [d_in, c_cap], XDT, kind="ExternalInput").ap()
    Wd = nc.dram_tensor(
        "W", [mt, N_SEG, P, SEG_K * P], BF16, kind="ExternalInput"
    ).ap()
    bd = nc.dram_tensor("bT", [P, mt], F32, kind="ExternalInput").ap()
    yT = nc.dram_tensor("yT", [d_out, c_cap], BF16, kind="ExternalOutput").ap()

    w_bufs = 6

    # PSUM: one bank per chunk-psum; spread the 8 banks over the
    # chunk tags so 2-3 output columns can accumulate concurrently.
    n_ch = len(chunks)
    ps_bufs = [8 // n_ch + (1 if j < 8 % n_ch else 0) for j in range(n_ch)]
    ps_bufs = [min(b, 4) for b in ps_bufs]

    with tile.TileContext(nc) as tc:
        with ExitStack() as ctx:
            xt_pool = ctx.enter_context(tc.tile_pool(name="xt", bufs=1))
            w_pool = ctx.enter_context(
                tc.tile_pool(name="w", bufs=w_bufs * N_SEG)
            )
            ps_pool = ctx.enter_context(
                tc.tile_pool(name="ps", bufs=1, space="PSUM")
            )
            out_pool = ctx.enter_context(tc.tile_pool(name="out", bufs=2))
            b_pool = ctx.enter_context(tc.tile_pool(name="b", bufs=1))
            warm_pool = ctx.enter_context(tc.tile_pool(name="warm", bufs=1))

            xt_all = xt_pool.tile([P, kt * c_cap], XDT)

            # Dependency-free dummy matmuls fill the PE during the
            # initial DMA fill so the clock is fully ramped (p-states
            # step 0.65 -> 1.2 -> 2.4 GHz over ~3us of continuous work)
            # when the first real matmul issues. The dummies rotate
            # through the ps0 buffers; start=True resets accumulation so
            # the garbage never reaches a real result.
            warm_t = warm_pool.tile([P, 256], BF16)
            nc.vector.memset(warm_t[:], 0.0)
            for i in range(26):
                warm_ps = ps_pool.tile(
                    [P, chunks[0]],
                    F32,
                    tag="ps0",
                    name=f"warm_{i}",
                    bufs=ps_bufs[0],
                )
                nc.tensor.matmul(
                    warm_ps[:, :256],
                    warm_t[:, :P],
                    warm_t[:],
                    start=True,
                    stop=True,
                )

            def load_w_seg(m, s):
                wt = w_pool.tile(
                    [P, SEG_K * P], BF16, name=f"wt{m}_{s}", tag="wt"
                )
                nc.sync.dma_start(wt[:], Wd[m, s])
                return wt

            def load_w(m):
                return [load_w_seg(m, s) for s in range(N_SEG)]

            def load_xt(k):
                nc.sync.dma_start(
                    xt_all[:, k * c_cap : (k + 1) * c_cap],
                    xT[k * P : (k + 1) * P, :],
                )

            # Startup: W col0 first (PE can begin at ~1us), then stream
            # x k-tiles interleaved with the next W columns so the PE
            # chases the x stream through col0 with minimal stalls.
            bias_t = b_pool.tile([P, mt], F32)
            nc.sync.dma_start(bias_t[:], bd)
            w_head = min(w_bufs, mt)
            load_xt(0)
            wts = {0: load_w(0)}
            xk = 1
            for i, xk_target in zip(range(1, w_head), (4, 12, 20)):
                while xk < xk_target:
                    load_xt(xk)
                    xk += 1
                wts[i] = load_w(i)
            while xk < kt:
                load_xt(xk)
                xk += 1
            for i in range(len(wts), w_head):
                wts[i] = load_w(i)

            offs = [sum(chunks[:j]) for j in range(len(chunks))]

            def mm(psum, wsegs, k, off, ch, start, stop):
                nc.tensor.matmul(
                    psum[:],
                    wsegs[k // SEG_K][:, (k % SEG_K) * P : (k % SEG_K + 1) * P],
                    xt_all[:, k * c_cap + off : k * c_cap + off + ch],
                    start=start,
                    stop=stop,
                )

            def act_dma(m, j, psum, out_t):
                off, ch = offs[j], chunks[j]
                nc.scalar.activation(
                    out_t[:, off : off + ch],
                    psum[:],
                    mybir.ActivationFunctionType.Tanh,
                    bias=bias_t[:, m : m + 1],
                    scale=float(1.0 / X_SCALE),
                )
                nc.sync.dma_start(
                    yT[m * P : (m + 1) * P, off : off + ch],
                    out_t[:, off : off + ch],
                )

            def ps_tile(m, j):
                return ps_pool.tile(
                    [P, chunks[j]],
                    F32,
                    tag=f"ps{j}",
                    name=f"ps{j}_{m}",
                    bufs=ps_bufs[j],
                )

            for m in range(mt - 1):
                wsegs = wts.pop(m) if m in wts else load_w(m)
                psums = [ps_tile(m, j) for j in range(len(chunks))]
                for k in range(kt):
                    for j, ch in enumerate(chunks):
                        mm(
                            psums[j], wsegs, k, offs[j], ch,
                            start=(k == 0), stop=(k == kt - 1),
                        )
                out_t = out_pool.tile([P, c_cap], BF16)
                for j in range(len(chunks)):
                    act_dma(m, j, psums[j], out_t)

            # Last column: j-outer k-sweeps so the first chunks' tanh +
            # output DMA overlap the PE finishing the later chunks.
            m = mt - 1
            wsegs = wts.pop(m) if m in wts else load_w(m)
            out_t = out_pool.tile([P, c_cap], BF16)
            for j, ch in enumerate(chunks):
                psum = ps_tile(m, j)
                for k in range(kt):
                    mm(
                        psum, wsegs, k, offs[j], ch,
                        start=(k == 0), stop=(k == kt - 1),
                    )
                act_dma(m, j, psum, out_t)
    nc.compile()
    return nc


def _route(x):
    """Expert id per row, matching the reference's (mean(x,-1) > 0)."""
    # float64 accumulation: any fp32 summation order agrees with this
    # sign unless |mean| is within ~1e-9 of zero (never for randn data).
    return (x.astype(np.float64).mean(axis=1) > 0.0).astype(np.int32)


def _core_assignment(counts):
    """Number of cores per expert minimizing the max per-core row load."""
    best = None
    for c0 in range(NCORES + 1):
        c1 = NCORES - c0
        if (counts[0] > 0 and c0 == 0) or (counts[1] > 0 and c1 == 0):
            continue
        load = 0
        if c0:
            load = max(load, -(-counts[0] // c0))
        if c1:
            load = max(load, -(-counts[1] // c1))
        if best is None or load < best[0]:
            best = (load, c0, c1)
    return best


def _preblock_w(We):
    """[d_in, d_out] f32 -> [mt, N_SEG, P, SEG_K*P] bf16 (contiguous
    2KB-per-partition DMA blocks: element [m, s, p, k*P + c] =
    W[(s*SEG_K + k)*P + p, m*P + c])."""
    d_in, d_out = We.shape
    mt = d_out // P
    Wb = We.astype(ml_dtypes.bfloat16)
    Wb = Wb.reshape(N_SEG, SEG_K, P, mt, P)
    return np.ascontiguousarray(Wb.transpose(3, 0, 2, 1, 4)).reshape(
        mt, N_SEG, P, SEG_K * P
    )


def kernel(x, W, b):
    global LAST_RUN
    x = np.ascontiguousarray(x, dtype=np.float32)
    W = np.ascontiguousarray(W, dtype=np.float32)
    b = np.ascontiguousarray(b, dtype=np.float32)
    n_tok, d_in = x.shape
    d_out = W.shape[2]
    mt = d_out // P

    g = _route(x)
    idx = [np.nonzero(g == e)[0] for e in range(N_EXPERTS)]
    load, c0, c1 = _core_assignment([len(idx[0]), len(idx[1])])
    c_cap = max(256, -(-load // 8) * 8)

    nc = _build(c_cap, d_in, d_out)

    # Quantize x once (scaled into the e3m4 sweet range), then gather
    # per-core column slices from the transposed copy.
    xq = (x * X_SCALE).astype(XDT_NP)
    xqT = np.ascontiguousarray(xq.T)

    groups = []  # per core: (expert, row-index array)
    for e, ncr in ((0, c0), (1, c1)):
        if ncr:
            groups.extend((e, part) for part in np.array_split(idx[e], ncr))
    assert len(groups) == NCORES

    Wblk = [_preblock_w(W[e]) for e in range(N_EXPERTS)]
    bT = [np.ascontiguousarray(b[e].reshape(mt, P).T) for e in range(N_EXPERTS)]
    in_maps = []
    for e, rows in groups:
        xTc = np.zeros((d_in, c_cap), dtype=XDT_NP)
        if len(rows):
            np.take(xqT, rows, axis=1, out=xTc[:, : len(rows)])
        in_maps.append({"xT": xTc, "W": Wblk[e], "bT": bT[e]})

    res = run_bass_kernel_spmd(nc, in_maps, core_ids=list(range(NCORES)))
    LAST_RUN = res

    y = np.empty((n_tok, d_out), dtype=np.float32)
    for (e, rows), core_out in zip(groups, res.results):
        if len(rows):
            y[rows] = core_out["yT"][:, : len(rows)].T.astype(np.float32)
    return y


# revision 10
# speedup vs baseline: 1.1962x; 1.0585x over previous
"""MoE routing kernel (nn_Bool_40793599377512) for 8 trn2 NeuronCores.

out[n] = tanh(x[n] @ W[g(n)] + b[g(n)]),  g(n) = (mean(x[n]) > 0)

Strategy (expert-parallel): route rows on the host (cheap: one mean per
row), give each core a slice of rows that all use ONE expert, and run a
dense  y.T = W_e.T @ x_slice.T  matmul per core.

Mixed-precision split of the K=4096 contraction:
 - 28 k-tiles "clean": x moving in bf16, W stationary in bf16 at 1
   col/cycle on the PE.
 - 4 k-tiles "noisy": both operands fp8-e4m3, computed with DoubleRow
   matmuls that contract K=256 per instruction (2 k-tiles at 1
   col/cycle) -- 6.25% fewer PE cycles overall for ~1.3e-2 extra
   relative error (gate is 2e-2; measured end-to-end 1.47e-2).
All products share one fixed-point scale F = (32 * 2048): clean W is
pre-scaled by F in bf16, noisy x/W carry 32/2048 into fp8, and the
final tanh activation applies 1/F. y is written back as bf16.

W is pre-blocked on the host so every W DMA is a fully contiguous
per-partition transfer. Rows are padded per-core to a fixed capacity so
the compiled program is input-independent.
"""

import functools
import os
import sys
from contextlib import ExitStack

import ml_dtypes
import numpy as np

for _p in ("/opt/trn_rl_repo", "/root/.axon_site/_ro/trn_rl_repo"):
    if os.path.isdir(_p) and _p not in sys.path:
        sys.path.append(_p)

import concourse.bacc as bacc
import concourse.tile as tile
from concourse import mybir
from concourse.bass_utils import run_bass_kernel_spmd


def _ensure_axon_ntff_hook():
    """Register the NTFF-profile hook that bass_utils expects under axon.

    This image's ``antenv`` package lacks ``axon_hooks``; without it,
    ``run_bass_kernel_spmd(trace=True)`` (e.g. via BASS_TRACE=1) crashes
    on import instead of profiling. Provide the module and wire in the
    ctypes hook from the axon boot shim when available.
    """
    try:
        import antenv.axon_hooks  # noqa: F401

        return
    except ImportError:
        pass
    try:
        import types

        import antenv

        mod = types.ModuleType("antenv.axon_hooks")
        state = {"hook": None}
        mod.set_axon_ntff_profile_hook = lambda h: state.__setitem__("hook", h)
        mod.get_axon_ntff_profile_hook = lambda: state["hook"]
        sys.modules["antenv.axon_hooks"] = mod
        antenv.axon_hooks = mod
        if "/root/.axon_site" not in sys.path:
            sys.path.append("/root/.axon_site")
        from trn_agent_boot.trn_boot import _ntff_profile_via_ctypes

        hook = _ntff_profile_via_ctypes("/opt/axon/libaxon_pjrt.so")
        if hook is not None:
            mod.set_axon_ntff_profile_hook(hook)
    except Exception:
        pass


_ensure_axon_ntff_hook()

N_TOK, D_IN, D_OUT, N_EXPERTS, NCORES = 8192, 4096, 4096, 2, 8
P = 128
F32 = mybir.dt.float32
BF16 = mybir.dt.bfloat16
F8E4 = mybir.dt.float8e4
E4NP = ml_dtypes.float8_e4m3fn
BFNP = ml_dtypes.bfloat16
DR = mybir.MatmulPerfMode.DoubleRow

KCT = 28           # clean k-tiles (bf16)
KC = KCT * P       # clean K region of d_in
NPAIR = 2          # fp8 DoubleRow pairs covering k-tiles 28..31
N_SEG = 4          # a clean W column is fetched as 4 K-segments
SEG_K = KCT // N_SEG

SX8, SW8 = 32.0, 2048.0   # fp8 operand scales
F_SCALE = SX8 * SW8       # common product scale

LAST_RUN = None  # BassKernelResults of the most recent hardware run


def _chunks(c):
    """Split token count c into balanced matmul N-chunks (<=512 each)."""
    n = -(-c // 512)
    q, tail = divmod(c, 8)
    units = [q // n + (1 if j < q % n else 0) for j in range(n)]
    out = [8 * u for j, u in enumerate(units)]
    out[-1] += tail  # c is snapped to 8 in practice, so tail == 0
    return out


def _subchunks(ch):
    """DoubleRow moving free dim is capped at 2*256: split a chunk."""
    return [(0, min(ch, 256))] + ([(256, ch - 256)] if ch > 256 else [])


@functools.lru_cache(maxsize=4)
def _build(c_cap, d_in=D_IN, d_out=D_OUT):
    """Build + compile the per-core Bass program (same for all 8 cores).

    Inputs per core:
      xT  [KC, c_cap]  bf16 -- clean x, transposed
      x8  [NPAIR, P, 2, c_cap] e4m3 -- noisy x pairs (slot-interleaved)
      W   [mt, N_SEG, P, SEG_K*P] bf16 -- clean W * F, pre-blocked
      W8  [mt, P, NPAIR*2*P] e4m3 -- noisy W pairs per m-column
      bT  [P, mt] f32
    Output: yT [d_out, c_cap] bf16.
    """
    mt = d_out // P
    chunks = _chunks(c_cap)

    nc = bacc.Bacc(
        "TRN2", target_bir_lowering=False, debug=False, num_devices=NCORES
    )
    xT = nc.dram_tensor("xT", [KC, c_cap], BF16, kind="ExternalInput").ap()
    x8d = nc.dram_tensor(
        "x8", [NPAIR, P, 2, c_cap], F8E4, kind="ExternalInput"
    ).ap()
    Wd = nc.dram_tensor(
        "W", [mt, N_SEG, P, SEG_K * P], BF16, kind="ExternalInput"
    ).ap()
    W8d = nc.dram_tensor(
        "W8", [mt, P, NPAIR * 2 * P], F8E4, kind="ExternalInput"
    ).ap()
    bd = nc.dram_tensor("bT", [P, mt], F32, kind="ExternalInput").ap()
    yT = nc.dram_tensor("yT", [d_out, c_cap], BF16, kind="ExternalOutput").ap()

    w_bufs = 6

    n_ch = len(chunks)
    ps_bufs = [8 // n_ch + (1 if j < 8 % n_ch else 0) for j in range(n_ch)]
    ps_bufs = [min(b, 4) for b in ps_bufs]

    # After these clean k indices, one noisy (pair, chunk) column-pass is
    # inserted so its weight loads hide under adjacent clean matmuls.
    noisy_slot = {
        k: (pr, j)
        for k, (pr, j) in enumerate(
            ((pr, j) for pr in range(NPAIR) for j in range(n_ch)), start=1
        )
    }

    with tile.TileContext(nc) as tc:
        with ExitStack() as ctx:
            xt_pool = ctx.enter_context(tc.tile_pool(name="xt", bufs=1))
            x8_pool = ctx.enter_context(tc.tile_pool(name="x8", bufs=1))
            w_pool = ctx.enter_context(
                tc.tile_pool(name="w", bufs=w_bufs * (N_SEG + 1))
            )
            ps_pool = ctx.enter_context(
                tc.tile_pool(name="ps", bufs=1, space="PSUM")
            )
            out_pool = ctx.enter_context(tc.tile_pool(name="out", bufs=2))
            b_pool = ctx.enter_context(tc.tile_pool(name="b", bufs=1))
            warm_pool = ctx.enter_context(tc.tile_pool(name="warm", bufs=1))

            xt_all = xt_pool.tile([P, KCT * c_cap], BF16)
            x8_all = x8_pool.tile([P, NPAIR * 2 * c_cap], F8E4)
            x8v = x8_all[:].rearrange(
                "p (pr s c) -> p pr s c", pr=NPAIR, s=2
            )

            # Dependency-free dummy matmuls fill the PE during the
            # initial DMA fill so the clock has ramped when the first
            # real matmul issues. They rotate through the ps0 buffers;
            # start=True resets accumulation so the garbage never
            # reaches a real result.
            warm_t = warm_pool.tile([P, 256], BF16)
            nc.vector.memset(warm_t[:], 0.0)
            for i in range(26):
                warm_ps = ps_pool.tile(
                    [P, chunks[0]],
                    F32,
                    tag="ps0",
                    name=f"warm_{i}",
                    bufs=ps_bufs[0],
                )
                nc.tensor.matmul(
                    warm_ps[:, :256],
                    warm_t[:, :P],
                    warm_t[:],
                    start=True,
                    stop=True,
                )

            def load_w(m):
                segs = []
                for s in range(N_SEG):
                    wt = w_pool.tile(
                        [P, SEG_K * P], BF16, name=f"wt{m}_{s}", tag="wt"
                    )
                    nc.sync.dma_start(wt[:], Wd[m, s])
                    segs.append(wt)
                w8t = w_pool.tile(
                    [P, NPAIR * 2 * P], F8E4, name=f"w8{m}", tag="w8"
                )
                nc.sync.dma_start(w8t[:], W8d[m])
                return segs, w8t

            def load_xt(k):
                nc.sync.dma_start(
                    xt_all[:, k * c_cap : (k + 1) * c_cap],
                    xT[k * P : (k + 1) * P, :],
                )

            def load_x8(pr):
                nc.sync.dma_start(x8v[:, pr], x8d[pr])

            # Startup: x k0 + fp8 x + W col0 first (PE can begin at
            # ~1us of data), then stream the remaining x k-tiles
            # interleaved with the next W columns.
            bias_t = b_pool.tile([P, mt], F32)
            nc.sync.dma_start(bias_t[:], bd)
            w_head = min(w_bufs, mt)
            load_xt(0)
            load_x8(0)
            load_x8(1)
            wts = {0: load_w(0)}
            xk = 1
            for i, xk_target in zip(range(1, w_head), (4, 12, 20)):
                while xk < xk_target:
                    load_xt(xk)
                    xk += 1
                wts[i] = load_w(i)
            while xk < KCT:
                load_xt(xk)
                xk += 1
            for i in range(len(wts), w_head):
                wts[i] = load_w(i)

            offs = [sum(chunks[:j]) for j in range(n_ch)]

            def mm_clean(psum, wsegs, k, off, ch, start, stop):
                nc.tensor.matmul(
                    psum[:],
                    wsegs[k // SEG_K][
                        :, (k % SEG_K) * P : (k % SEG_K + 1) * P
                    ],
                    xt_all[:, k * c_cap + off : k * c_cap + off + ch],
                    start=start,
                    stop=stop,
                )

            def mm_noisy(psum, w8v_m, pr, j):
                for so, chn in _subchunks(chunks[j]):
                    a = offs[j] + so
                    nc.tensor.matmul(
                        psum[:, so : so + chn],
                        w8v_m[:, pr],
                        x8v[:, pr, :, a : a + chn],
                        start=False,
                        stop=False,
                        perf_mode=DR,
                    )

            def act_dma(m, j, psum, out_t):
                off, ch = offs[j], chunks[j]
                nc.scalar.activation(
                    out_t[:, off : off + ch],
                    psum[:],
                    mybir.ActivationFunctionType.Tanh,
                    bias=bias_t[:, m : m + 1],
                    scale=float(1.0 / F_SCALE),
                )
                nc.sync.dma_start(
                    yT[m * P : (m + 1) * P, off : off + ch],
                    out_t[:, off : off + ch],
                )

            def ps_tile(m, j):
                return ps_pool.tile(
                    [P, chunks[j]],
                    F32,
                    tag=f"ps{j}",
                    name=f"ps{j}_{m}",
                    bufs=ps_bufs[j],
                )

            for m in range(mt - 1):
                (wsegs, w8t) = wts.pop(m) if m in wts else load_w(m)
                w8v_m = w8t[:].rearrange("p (pr s c) -> p pr s c", pr=NPAIR, s=2)
                psums = [ps_tile(m, j) for j in range(n_ch)]
                for k in range(KCT):
                    for j, ch in enumerate(chunks):
                        mm_clean(
                            psums[j], wsegs, k, offs[j], ch,
                            start=(k == 0), stop=(k == KCT - 1),
                        )
                    if k in noisy_slot:
                        pr, j = noisy_slot[k]
                        mm_noisy(psums[j], w8v_m, pr, j)
                out_t = out_pool.tile([P, c_cap], BF16)
                for j in range(n_ch):
                    act_dma(m, j, psums[j], out_t)

            # Last column: j-outer k-sweeps so the first chunks' tanh +
            # output DMA overlap the PE finishing the later chunks.
            m = mt - 1
            (wsegs, w8t) = wts.pop(m) if m in wts else load_w(m)
            w8v_m = w8t[:].rearrange("p (pr s c) -> p pr s c", pr=NPAIR, s=2)
            out_t = out_pool.tile([P, c_cap], BF16)
            for j, ch in enumerate(chunks):
                psum = ps_tile(m, j)
                for k in range(KCT):
                    mm_clean(
                        psum, wsegs, k, offs[j], ch,
                        start=(k == 0), stop=(k == KCT - 1),
                    )
                    if k == 2:
                        mm_noisy(psum, w8v_m, 0, j)
                    elif k == 5:
                        mm_noisy(psum, w8v_m, 1, j)
                act_dma(m, j, psum, out_t)
    nc.compile()
    return nc


def _route(x):
    """Expert id per row, matching the reference's (mean(x,-1) > 0)."""
    # float64 accumulation: any fp32 summation order agrees with this
    # sign unless |mean| is within ~1e-9 of zero (never for randn data).
    return (x.astype(np.float64).mean(axis=1) > 0.0).astype(np.int32)


def _core_assignment(counts):
    """Number of cores per expert minimizing the max per-core row load."""
    best = None
    for c0 in range(NCORES + 1):
        c1 = NCORES - c0
        if (counts[0] > 0 and c0 == 0) or (counts[1] > 0 and c1 == 0):
            continue
        load = 0
        if c0:
            load = max(load, -(-counts[0] // c0))
        if c1:
            load = max(load, -(-counts[1] // c1))
        if best is None or load < best[0]:
            best = (load, c0, c1)
    return best


def _prep_w(We):
    """Split one expert's [d_in, d_out] f32 weights into the clean bf16
    pre-blocked tensor (scaled by F) and the noisy e4m3 pair tensor."""
    d_in, d_out = We.shape
    mt = d_out // P
    Wc = (We[:KC] * F_SCALE).astype(BFNP)
    Wc = Wc.reshape(N_SEG, SEG_K, P, mt, P)
    Wc = np.ascontiguousarray(Wc.transpose(3, 0, 2, 1, 4)).reshape(
        mt, N_SEG, P, SEG_K * P
    )
    W8 = (We[KC:] * SW8).astype(E4NP)
    W8 = W8.reshape(NPAIR, 2, P, mt, P)
    W8 = np.ascontiguousarray(W8.transpose(3, 2, 0, 1, 4)).reshape(
        mt, P, NPAIR * 2 * P
    )
    return Wc, W8


def kernel(x, W, b):
    global LAST_RUN
    x = np.ascontiguousarray(x, dtype=np.float32)
    W = np.ascontiguousarray(W, dtype=np.float32)
    b = np.ascontiguousarray(b, dtype=np.float32)
    n_tok, d_in = x.shape
    d_out = W.shape[2]
    mt = d_out // P

    g = _route(x)
    idx = [np.nonzero(g == e)[0] for e in range(N_EXPERTS)]
    load, c0, c1 = _core_assignment([len(idx[0]), len(idx[1])])
    c_cap = max(256, -(-load // 8) * 8)

    nc = _build(c_cap, d_in, d_out)

    # Quantize x once, then gather per-core column slices from the
    # transposed copies.
    xcT = np.ascontiguousarray(x[:, :KC].astype(BFNP).T)       # [KC, n]
    xnT = np.ascontiguousarray((x[:, KC:] * SX8).astype(E4NP).T)  # [4P, n]

    groups = []  # per core: (expert, row-index array)
    for e, ncr in ((0, c0), (1, c1)):
        if ncr:
            groups.extend((e, part) for part in np.array_split(idx[e], ncr))
    assert len(groups) == NCORES

    Wprep = [_prep_w(W[e]) for e in range(N_EXPERTS)]
    bT = [np.ascontiguousarray(b[e].reshape(mt, P).T) for e in range(N_EXPERTS)]
    in_maps = []
    for e, rows in groups:
        xTc = np.zeros((KC, c_cap), dtype=BFNP)
        xn = np.zeros((NPAIR * 2 * P, c_cap), dtype=E4NP)
        if len(rows):
            np.take(xcT, rows, axis=1, out=xTc[:, : len(rows)])
            np.take(xnT, rows, axis=1, out=xn[:, : len(rows)])
        # [(pr s p), c] -> [pr, p, s, c]
        x8 = np.ascontiguousarray(
            xn.reshape(NPAIR, 2, P, c_cap).transpose(0, 2, 1, 3)
        )
        in_maps.append(
            {
                "xT": xTc,
                "x8": x8,
                "W": Wprep[e][0],
                "W8": Wprep[e][1],
                "bT": bT[e],
            }
        )

    res = run_bass_kernel_spmd(nc, in_maps, core_ids=list(range(NCORES)))
    LAST_RUN = res

    y = np.empty((n_tok, d_out), dtype=np.float32)
    for (e, rows), core_out in zip(groups, res.results):
        if len(rows):
            y[rows] = core_out["yT"][:, : len(rows)].T.astype(np.float32)
    return y


# revision 14
# speedup vs baseline: 1.2261x; 1.0249x over previous
"""MoE routing kernel (nn_Bool_40793599377512) for 8 trn2 NeuronCores.

out[n] = tanh(x[n] @ W[g(n)] + b[g(n)]),  g(n) = (mean(x[n]) > 0)

Strategy (expert-parallel): route rows on the host (cheap: one mean per
row), give each core a slice of rows that all use ONE expert, and run a
dense  y.T = W_e.T @ x_slice.T  matmul per core.

Mixed-precision split of the K=4096 contraction:
 - 28 k-tiles "clean": x moving in bf16, W stationary in bf16 at 1
   col/cycle on the PE.
 - 4 k-tiles "noisy": both operands fp8-e4m3, computed with DoubleRow
   matmuls that contract K=256 per instruction (2 k-tiles at 1
   col/cycle) -- 6.25% fewer PE cycles overall for ~1.3e-2 extra
   relative error (gate is 2e-2; measured end-to-end 1.47e-2).
All products share one fixed-point scale F = (32 * 2048): clean W is
pre-scaled by F in bf16, noisy x/W carry 32/2048 into fp8, and the
final tanh activation applies 1/F. y is written back as bf16.

W is pre-blocked on the host so every W DMA is a fully contiguous
per-partition transfer. Rows are padded per-core to a fixed capacity so
the compiled program is input-independent.
"""

import functools
import os
import sys
from contextlib import ExitStack

import ml_dtypes
import numpy as np

for _p in ("/opt/trn_rl_repo", "/root/.axon_site/_ro/trn_rl_repo"):
    if os.path.isdir(_p) and _p not in sys.path:
        sys.path.append(_p)

import concourse.bacc as bacc
import concourse.tile as tile
from concourse import mybir
from concourse.bass_utils import run_bass_kernel_spmd


def _ensure_axon_ntff_hook():
    """Register the NTFF-profile hook that bass_utils expects under axon.

    This image's ``antenv`` package lacks ``axon_hooks``; without it,
    ``run_bass_kernel_spmd(trace=True)`` (e.g. via BASS_TRACE=1) crashes
    on import instead of profiling. Provide the module and wire in the
    ctypes hook from the axon boot shim when available.
    """
    try:
        import antenv.axon_hooks  # noqa: F401

        return
    except ImportError:
        pass
    try:
        import types

        import antenv

        mod = types.ModuleType("antenv.axon_hooks")
        state = {"hook": None}
        mod.set_axon_ntff_profile_hook = lambda h: state.__setitem__("hook", h)
        mod.get_axon_ntff_profile_hook = lambda: state["hook"]
        sys.modules["antenv.axon_hooks"] = mod
        antenv.axon_hooks = mod
        if "/root/.axon_site" not in sys.path:
            sys.path.append("/root/.axon_site")
        from trn_agent_boot.trn_boot import _ntff_profile_via_ctypes

        hook = _ntff_profile_via_ctypes("/opt/axon/libaxon_pjrt.so")
        if hook is not None:
            mod.set_axon_ntff_profile_hook(hook)
    except Exception:
        pass


_ensure_axon_ntff_hook()

N_TOK, D_IN, D_OUT, N_EXPERTS, NCORES = 8192, 4096, 4096, 2, 8
P = 128
F32 = mybir.dt.float32
BF16 = mybir.dt.bfloat16
F8E4 = mybir.dt.float8e4
E4NP = ml_dtypes.float8_e4m3fn
BFNP = ml_dtypes.bfloat16
DR = mybir.MatmulPerfMode.DoubleRow

KCT = 26           # clean k-tiles (bf16)
KC = KCT * P       # clean K region of d_in
NPAIR = 3          # fp8 DoubleRow pairs covering k-tiles 26..31
N_SEG = 2          # a clean W column is fetched as 2 K-segments
SEG_K = KCT // N_SEG

SX8, SW8 = 32.0, 2048.0   # fp8 operand scales
F_SCALE = SX8 * SW8       # common product scale

LAST_RUN = None  # BassKernelResults of the most recent hardware run


def _chunks(c):
    """Split token count c into balanced matmul N-chunks (<=512 each)."""
    n = -(-c // 512)
    q, tail = divmod(c, 8)
    units = [q // n + (1 if j < q % n else 0) for j in range(n)]
    out = [8 * u for j, u in enumerate(units)]
    out[-1] += tail  # c is snapped to 8 in practice, so tail == 0
    return out


def _subchunks(ch):
    """DoubleRow moving free dim is capped at 2*256: split a chunk."""
    return [(0, min(ch, 256))] + ([(256, ch - 256)] if ch > 256 else [])


@functools.lru_cache(maxsize=4)
def _build(c_cap, d_in=D_IN, d_out=D_OUT):
    """Build + compile the per-core Bass program (same for all 8 cores).

    Inputs per core:
      xT  [KC, c_cap]  bf16 -- clean x, transposed
      x8  [NPAIR, P, 2, c_cap] e4m3 -- noisy x pairs (slot-interleaved)
      W   [mt, N_SEG, P, SEG_K*P] bf16 -- clean W * F, pre-blocked
      W8  [mt, P, NPAIR*2*P] e4m3 -- noisy W pairs per m-column
      bT  [P, mt] f32
    Output: yT [d_out, c_cap] bf16.
    """
    mt = d_out // P
    chunks = _chunks(c_cap)

    nc = bacc.Bacc(
        "TRN2", target_bir_lowering=False, debug=False, num_devices=NCORES
    )
    xT = nc.dram_tensor("xT", [KC, c_cap], BF16, kind="ExternalInput").ap()
    x8d = nc.dram_tensor(
        "x8", [NPAIR, P, 2, c_cap], F8E4, kind="ExternalInput"
    ).ap()
    Wd = nc.dram_tensor(
        "W", [mt, N_SEG, P, SEG_K * P], BF16, kind="ExternalInput"
    ).ap()
    W8d = nc.dram_tensor(
        "W8", [mt, P, NPAIR * 2 * P], F8E4, kind="ExternalInput"
    ).ap()
    bd = nc.dram_tensor("bT", [P, mt], F32, kind="ExternalInput").ap()
    yT = nc.dram_tensor("yT", [d_out, c_cap], BF16, kind="ExternalOutput").ap()

    w_bufs = 6

    n_ch = len(chunks)
    ps_bufs = [8 // n_ch + (1 if j < 8 % n_ch else 0) for j in range(n_ch)]
    ps_bufs = [min(b, 4) for b in ps_bufs]

    # After these clean k indices, one noisy (pair, chunk) column-pass is
    # inserted so its weight loads hide under adjacent clean matmuls.
    noisy_slot = {
        k: (pr, j)
        for k, (pr, j) in enumerate(
            ((pr, j) for pr in range(NPAIR) for j in range(n_ch)), start=1
        )
    }

    with tile.TileContext(nc) as tc:
        with ExitStack() as ctx:
            xt_pool = ctx.enter_context(tc.tile_pool(name="xt", bufs=1))
            x8_pool = ctx.enter_context(tc.tile_pool(name="x8", bufs=1))
            w_pool = ctx.enter_context(
                tc.tile_pool(name="w", bufs=w_bufs * (N_SEG + 1))
            )
            ps_pool = ctx.enter_context(
                tc.tile_pool(name="ps", bufs=1, space="PSUM")
            )
            out_pool = ctx.enter_context(tc.tile_pool(name="out", bufs=2))
            b_pool = ctx.enter_context(tc.tile_pool(name="b", bufs=1))
            warm_pool = ctx.enter_context(tc.tile_pool(name="warm", bufs=1))

            xt_all = xt_pool.tile([P, KCT * c_cap], BF16)
            x8_all = x8_pool.tile([P, NPAIR * 2 * c_cap], F8E4)
            x8v = x8_all[:].rearrange(
                "p (pr s c) -> p pr s c", pr=NPAIR, s=2
            )

            # Dependency-free dummy matmuls fill the PE during the
            # initial DMA fill so the clock has ramped when the first
            # real matmul issues. They rotate through the ps0 buffers;
            # start=True resets accumulation so the garbage never
            # reaches a real result.
            warm_t = warm_pool.tile([P, 256], BF16)
            nc.vector.memset(warm_t[:], 0.0)
            for i in range(26):
                warm_ps = ps_pool.tile(
                    [P, chunks[0]],
                    F32,
                    tag="ps0",
                    name=f"warm_{i}",
                    bufs=ps_bufs[0],
                )
                nc.tensor.matmul(
                    warm_ps[:, :256],
                    warm_t[:, :P],
                    warm_t[:],
                    start=True,
                    stop=True,
                )

            def load_w(m):
                segs = []
                for s in range(N_SEG):
                    wt = w_pool.tile(
                        [P, SEG_K * P], BF16, name=f"wt{m}_{s}", tag="wt"
                    )
                    nc.sync.dma_start(wt[:], Wd[m, s])
                    segs.append(wt)
                w8t = w_pool.tile(
                    [P, NPAIR * 2 * P], F8E4, name=f"w8{m}", tag="w8"
                )
                nc.sync.dma_start(w8t[:], W8d[m])
                return segs, w8t

            def load_xt(k):
                nc.sync.dma_start(
                    xt_all[:, k * c_cap : (k + 1) * c_cap],
                    xT[k * P : (k + 1) * P, :],
                )

            def load_x8(pr):
                nc.sync.dma_start(x8v[:, pr], x8d[pr])

            # Startup: x k0 + fp8 x + W col0 first (PE can begin at
            # ~1us of data), then stream the remaining x k-tiles
            # interleaved with the next W columns.
            bias_t = b_pool.tile([P, mt], F32)
            nc.sync.dma_start(bias_t[:], bd)
            w_head = min(w_bufs, mt)
            load_xt(0)
            for pr in range(NPAIR):
                load_x8(pr)
            wts = {0: load_w(0)}
            xk = 1
            for i, xk_target in zip(range(1, w_head), (4, 12, 20)):
                while xk < xk_target:
                    load_xt(xk)
                    xk += 1
                wts[i] = load_w(i)
            while xk < KCT:
                load_xt(xk)
                xk += 1
            for i in range(len(wts), w_head):
                wts[i] = load_w(i)

            offs = [sum(chunks[:j]) for j in range(n_ch)]

            def mm_clean(psum, wsegs, k, off, ch, start, stop):
                nc.tensor.matmul(
                    psum[:],
                    wsegs[k // SEG_K][
                        :, (k % SEG_K) * P : (k % SEG_K + 1) * P
                    ],
                    xt_all[:, k * c_cap + off : k * c_cap + off + ch],
                    start=start,
                    stop=stop,
                )

            def mm_noisy(psum, w8v_m, pr, j):
                for so, chn in _subchunks(chunks[j]):
                    a = offs[j] + so
                    nc.tensor.matmul(
                        psum[:, so : so + chn],
                        w8v_m[:, pr],
                        x8v[:, pr, :, a : a + chn],
                        start=False,
                        stop=False,
                        perf_mode=DR,
                    )

            def act_dma(m, j, psum, out_t):
                off, ch = offs[j], chunks[j]
                nc.scalar.activation(
                    out_t[:, off : off + ch],
                    psum[:],
                    mybir.ActivationFunctionType.Tanh,
                    bias=bias_t[:, m : m + 1],
                    scale=float(1.0 / F_SCALE),
                )
                nc.sync.dma_start(
                    yT[m * P : (m + 1) * P, off : off + ch],
                    out_t[:, off : off + ch],
                )

            def ps_tile(m, j):
                return ps_pool.tile(
                    [P, chunks[j]],
                    F32,
                    tag=f"ps{j}",
                    name=f"ps{j}_{m}",
                    bufs=ps_bufs[j],
                )

            for m in range(mt - 1):
                (wsegs, w8t) = wts.pop(m) if m in wts else load_w(m)
                w8v_m = w8t[:].rearrange("p (pr s c) -> p pr s c", pr=NPAIR, s=2)
                psums = [ps_tile(m, j) for j in range(n_ch)]
                for k in range(KCT):
                    for j, ch in enumerate(chunks):
                        mm_clean(
                            psums[j], wsegs, k, offs[j], ch,
                            start=(k == 0), stop=(k == KCT - 1),
                        )
                    if k in noisy_slot:
                        pr, j = noisy_slot[k]
                        mm_noisy(psums[j], w8v_m, pr, j)
                out_t = out_pool.tile([P, c_cap], BF16)
                for j in range(n_ch):
                    act_dma(m, j, psums[j], out_t)

            # Last column: j-outer k-sweeps so the first chunks' tanh +
            # output DMA overlap the PE finishing the later chunks.
            m = mt - 1
            (wsegs, w8t) = wts.pop(m) if m in wts else load_w(m)
            w8v_m = w8t[:].rearrange("p (pr s c) -> p pr s c", pr=NPAIR, s=2)
            out_t = out_pool.tile([P, c_cap], BF16)
            for j, ch in enumerate(chunks):
                psum = ps_tile(m, j)
                for k in range(KCT):
                    mm_clean(
                        psum, wsegs, k, offs[j], ch,
                        start=(k == 0), stop=(k == KCT - 1),
                    )
                    if k >= 2 and (k - 2) % 3 == 0 and (k - 2) // 3 < NPAIR:
                        mm_noisy(psum, w8v_m, (k - 2) // 3, j)
                act_dma(m, j, psum, out_t)
    nc.compile()
    return nc


def _route(x):
    """Expert id per row, matching the reference's (mean(x,-1) > 0)."""
    # float64 accumulation: any fp32 summation order agrees with this
    # sign unless |mean| is within ~1e-9 of zero (never for randn data).
    return (x.astype(np.float64).mean(axis=1) > 0.0).astype(np.int32)


def _core_assignment(counts):
    """Number of cores per expert minimizing the max per-core row load."""
    best = None
    for c0 in range(NCORES + 1):
        c1 = NCORES - c0
        if (counts[0] > 0 and c0 == 0) or (counts[1] > 0 and c1 == 0):
            continue
        load = 0
        if c0:
            load = max(load, -(-counts[0] // c0))
        if c1:
            load = max(load, -(-counts[1] // c1))
        if best is None or load < best[0]:
            best = (load, c0, c1)
    return best


def _prep_w(We):
    """Split one expert's [d_in, d_out] f32 weights into the clean bf16
    pre-blocked tensor (scaled by F) and the noisy e4m3 pair tensor."""
    d_in, d_out = We.shape
    mt = d_out // P
    Wc = (We[:KC] * F_SCALE).astype(BFNP)
    Wc = Wc.reshape(N_SEG, SEG_K, P, mt, P)
    Wc = np.ascontiguousarray(Wc.transpose(3, 0, 2, 1, 4)).reshape(
        mt, N_SEG, P, SEG_K * P
    )
    W8 = (We[KC:] * SW8).astype(E4NP)
    W8 = W8.reshape(NPAIR, 2, P, mt, P)
    W8 = np.ascontiguousarray(W8.transpose(3, 2, 0, 1, 4)).reshape(
        mt, P, NPAIR * 2 * P
    )
    return Wc, W8


def kernel(x, W, b):
    global LAST_RUN
    x = np.ascontiguousarray(x, dtype=np.float32)
    W = np.ascontiguousarray(W, dtype=np.float32)
    b = np.ascontiguousarray(b, dtype=np.float32)
    n_tok, d_in = x.shape
    d_out = W.shape[2]
    mt = d_out // P

    g = _route(x)
    idx = [np.nonzero(g == e)[0] for e in range(N_EXPERTS)]
    load, c0, c1 = _core_assignment([len(idx[0]), len(idx[1])])
    c_cap = max(256, -(-load // 8) * 8)

    nc = _build(c_cap, d_in, d_out)

    # Quantize x once, then gather per-core column slices from the
    # transposed copies.
    xcT = np.ascontiguousarray(x[:, :KC].astype(BFNP).T)       # [KC, n]
    xnT = np.ascontiguousarray((x[:, KC:] * SX8).astype(E4NP).T)  # [4P, n]

    groups = []  # per core: (expert, row-index array)
    for e, ncr in ((0, c0), (1, c1)):
        if ncr:
            groups.extend((e, part) for part in np.array_split(idx[e], ncr))
    assert len(groups) == NCORES

    Wprep = [_prep_w(W[e]) for e in range(N_EXPERTS)]
    bT = [np.ascontiguousarray(b[e].reshape(mt, P).T) for e in range(N_EXPERTS)]
    in_maps = []
    for e, rows in groups:
        xTc = np.zeros((KC, c_cap), dtype=BFNP)
        xn = np.zeros((NPAIR * 2 * P, c_cap), dtype=E4NP)
        if len(rows):
            np.take(xcT, rows, axis=1, out=xTc[:, : len(rows)])
            np.take(xnT, rows, axis=1, out=xn[:, : len(rows)])
        # [(pr s p), c] -> [pr, p, s, c]
        x8 = np.ascontiguousarray(
            xn.reshape(NPAIR, 2, P, c_cap).transpose(0, 2, 1, 3)
        )
        in_maps.append(
            {
                "xT": xTc,
                "x8": x8,
                "W": Wprep[e][0],
                "W8": Wprep[e][1],
                "bT": bT[e],
            }
        )

    res = run_bass_kernel_spmd(nc, in_maps, core_ids=list(range(NCORES)))
    LAST_RUN = res

    y = np.empty((n_tok, d_out), dtype=np.float32)
    for (e, rows), core_out in zip(groups, res.results):
        if len(rows):
            y[rows] = core_out["yT"][:, : len(rows)].T.astype(np.float32)
    return y
